# revision 23
# baseline (speedup 1.0000x reference)
"""GraphSAGE 2-layer fraud detector on 8 trn2 NeuronCores.

Strategy (dst-partitioned, matmul scatter, minimal host->device traffic):
  - The axon tunnel moves ~45MB/s, so wall time is dominated by (a) input
    bytes shipped per spmd call and (b) per-call re-lowering of the kernel
    BIR (proportional to instruction count). Each core receives ONLY its
    x shard, quantized to int8 with a per-node fp16 scale (0.8MB), plus
    compressed edge tables; x is AllGathered across cores on-device, and
    everything else (iota, identity, x^T blocks, the z table) is derived
    on-device. All loops are tc.For_i hardware loops, so the kernel is a
    few hundred instructions regardless of edge count.
  - Nodes padded to 50176 = 8 cores x 49 blocks x 128. Core c owns nodes
    [c*6272, (c+1)*6272). Within a core, dst block b holds the 128 nodes
    with local index p*49 + b (p = row in block), which makes the z tile a
    plain contiguous view of z rows in node order.
  - Per-edge work is driven by gpsimd.dma_gather: one instruction gathers
    a whole block's 256B rows from an HBM table into SBUF. Rows pack two
    consecutive nodes (int8 x: 2x128B; z: 2x2 fp16 values in a padded
    row), so indices are src>>1 and fit int16. The parity selection AND
    the int8 dequant scale are folded into the one-hot scatter matrices:
      agg = sum_k [(iota==ldst_k)*sclE_k].T @ q_even
                + [(iota==ldst_k)*sclO_k].T @ q_odd
    where sclE/sclO = scale[src] masked by src parity (one fused
    tensor_scalar builds each matrix). Layer 2 uses the SAME index/ldst
    tables with parity masks instead of scales.
  - z = h@W2l.T, o = h@W2r.T + b2 (aggregation commutes with the linear
    map, so layer 2 aggregates 2-wide z, not 256-wide h); out =
    recip*agg2 + o.
"""

import time

import numpy as np

import concourse.bass as bass
import concourse.mybir as mybir
import concourse.tile as tile
from concourse import bacc
from concourse.bass import ds, ts
from concourse.bass_utils import run_bass_kernel_spmd

# generate_dve_tables() is a pure function of (trn_type, ops) but is re-run
# from scratch inside every run_bass_kernel_spmd call (~0.3s of deepcopy per
# call via neuronx_cc_hook -> compile_bir_kernel -> get_walrus_args).
# Memoize the common (ops == {}) case; the returned dict[str, bytes] is only
# ever read (write_dve_dir copies it to disk), so sharing one instance is safe.
import concourse.bass_utils as _bass_utils
import concourse.dve_table_gen as _dtg

_DVE_TABLE_CACHE: dict = {}
_orig_generate_dve_tables = _dtg.generate_dve_tables


def _cached_generate_dve_tables(trn_type, ops, base_dir=None):
    if ops or base_dir is not None:
        return _orig_generate_dve_tables(trn_type, ops, base_dir)
    if trn_type not in _DVE_TABLE_CACHE:
        _DVE_TABLE_CACHE[trn_type] = _orig_generate_dve_tables(trn_type, ops)
    return _DVE_TABLE_CACHE[trn_type]


_bass_utils.generate_dve_tables = _cached_generate_dve_tables
_dtg.generate_dve_tables = _cached_generate_dve_tables

# neuronx_cc_hook is likewise a pure function of the serialized HLO (which
# embeds the BIR), but each run_bass_kernel_spmd call re-runs walrus + NEFF
# tar repacking (~70ms) because every call makes a fresh jax.jit closure.
# Memoize per HLO bytes; the cached value is an immutable (rc, bytes) tuple.
import concourse.bass2jax as _b2j

_NEFF_HOOK_CACHE: dict = {}
_orig_neuronx_cc_hook = _b2j.neuronx_cc_hook


def _cached_neuronx_cc_hook(code, code_format, platform_version, file_prefix):
    if b"bass_exec" not in code:
        return _orig_neuronx_cc_hook(code, code_format, platform_version,
                                     file_prefix)
    key = (code, code_format, str(platform_version))
    r = _NEFF_HOOK_CACHE.get(key)
    if r is None:
        r = _orig_neuronx_cc_hook(code, code_format, platform_version,
                                  file_prefix)
        _NEFF_HOOK_CACHE[key] = r
    return r


_b2j.neuronx_cc_hook = _cached_neuronx_cc_hook

# Let XLA reuse compiled executables across the per-call fresh jit closures
# (harmless no-op if the backend doesn't support serialization).
try:
    import jax as _jax

    _jax.config.update("jax_compilation_cache_dir", "/tmp/jax_comp_cache")
    _jax.config.update("jax_persistent_cache_min_compile_time_secs", 0.0)
    _jax.config.update("jax_persistent_cache_min_entry_size_bytes", 0)
except Exception:
    pass

N = 50000
E = 800000
IN_C = 128
HID = 256
OUT_C = 2
NCORES = 8
P = 128
NB = 49                 # dst blocks per core
ROWS = NB * P           # 6272 rows per core
NP = NCORES * ROWS      # 50176 padded nodes
HNP = NP // 2           # 25088 paired rows (int16-addressable)

f32 = mybir.dt.float32
f16 = mybir.dt.float16
i32 = mybir.dt.int32
i16 = mybir.dt.int16
i8 = mybir.dt.int8
u8 = mybir.dt.uint8


def _wrap16(flat):
    """dma_gather index layout: flat j -> [partition j%16, col j//16]."""
    return np.ascontiguousarray(flat.reshape(-1, 16).T)


def _host_prep(x, edge_index, W1l, b1, W1r, W2l, b2, W2r):
    src = edge_index[0].astype(np.int64)
    dst = edge_index[1].astype(np.int64)
    cnt = np.bincount(dst, minlength=NP)
    recip = (1.0 / np.maximum(cnt, 1)).astype(np.float32)

    # int8 quantization of x with per-node fp16 scale
    x = np.asarray(x, np.float32)
    absmax = np.abs(x).max(axis=1)
    s_node = (np.maximum(absmax, 1e-6) / 127.0).astype(np.float16)
    s_full = np.ones(NP, np.float16)
    s_full[:N] = s_node
    q = np.zeros((NP, IN_C), np.int8)
    q[:N] = np.clip(np.rint(x / s_node.astype(np.float32)[:, None]),
                    -127, 127).astype(np.int8)

    # dst sort key in block-layout space: node (core c, local r) sits in
    # block b = r % 49 at row p = r // 49 -> key = c*6272 + b*128 + p.
    c_ = dst // ROWS
    r_ = dst % ROWS
    key = c_ * ROWS + (r_ % NB) * P + (r_ // NB)
    order = np.argsort(key, kind="stable")
    s_src = src[order]
    s_key = key[order]

    block_starts = np.searchsorted(s_key, np.arange(0, NP + P, P))
    cnt_blk = block_starts[1:] - block_starts[:-1]
    W = int(np.maximum(1, -(-cnt_blk // P)).max())  # uniform chunks per block
    C1 = NB * W

    # scale codes: 0 = pad (kills the edge in layer 1), else s = code*sstep
    smax = float(s_full[:N].max()) if N else 1.0
    sstep = smax / 255.0
    idx_arr = np.zeros((NCORES, 16, NB * 8 * W), np.int16)
    lp_arr = np.zeros((NCORES, P, C1), np.uint8)   # par*128 + ldst
    scl_arr = np.zeros((NCORES, P, C1), np.uint8)
    for c in range(NCORES):
        for b in range(NB):
            bb = c * NB + b
            s, e = int(block_starts[bb]), int(block_starts[bb + 1])
            k = e - s
            bs = s_src[s:e]
            fi = np.full(W * P, HNP, np.int16)   # pad -> zero row
            fi[:k] = bs >> 1
            idx_arr[c, :, b * 8 * W:(b + 1) * 8 * W] = _wrap16(fi)
            tl = np.zeros(W * P, np.uint8)
            tl[:k] = ((s_key[s:e] % P)
                      + 128 * (bs & 1)).astype(np.uint8)
            lp_arr[c, :, b * W:(b + 1) * W] = tl.reshape(W, P).T
            tsc = np.zeros(W * P, np.uint8)
            tsc[:k] = np.maximum(
                1, np.rint(s_full[bs].astype(np.float64) / sstep)
            ).astype(np.uint8)
            scl_arr[c, :, b * W:(b + 1) * W] = tsc.reshape(W, P).T

    W1lT = np.ascontiguousarray(W1l.T.astype(np.float16))   # [128, 256]
    W1rT = np.ascontiguousarray(W1r.T.astype(np.float16))
    Wzo = np.zeros((P, 8), np.float16)
    for j in range(2):
        Wzo[:, 4 * j:4 * j + 2] = W2l.T[j * P:(j + 1) * P, :].astype(np.float16)
        Wzo[:, 4 * j + 2:4 * j + 4] = W2r.T[j * P:(j + 1) * P, :].astype(np.float16)
    b1p = np.ascontiguousarray(np.asarray(b1).reshape(2, P).T.astype(np.float32))
    b2b = np.tile(np.asarray(b2).reshape(1, 2), (P, 1)).astype(np.float32)
    recip_c = recip.reshape(NCORES, P, NB).copy()   # node local r = p*49+b
    s_own = s_full.astype(np.float32).reshape(NCORES, P, NB)

    in_maps = []
    for c in range(NCORES):
        sections = [
            np.ascontiguousarray(idx_arr[c]),
            np.ascontiguousarray(lp_arr[c]),
            np.ascontiguousarray(scl_arr[c]),
            np.ascontiguousarray(s_own[c]),
            W1lT, W1rT, Wzo, b1p, b2b,
            np.ascontiguousarray(recip_c[c]),
        ]
        parts = []
        off = 0
        for a in sections:
            bb_ = a.ravel().view(np.uint8)
            parts.append(bb_)
            off += len(bb_)
            pad = (-off) % 256
            if pad:
                parts.append(np.zeros(pad, np.uint8))
                off += pad
        in_maps.append({
            "x_q": np.ascontiguousarray(
                q[c * ROWS:(c + 1) * ROWS, :].reshape(ROWS // 2, 2 * IN_C)),
            "tb": np.concatenate(parts)[None, :],
        })
    return in_maps, W, sstep


def _blob_offsets(W):
    C1 = NB * W
    sizes = [
        16 * NB * 8 * W * 2,    # idx16
        P * C1,                 # lpu
        P * C1,                 # sclu
        P * NB * 4,             # sclown f32
        P * HID * 2,            # W1lT f16
        P * HID * 2,            # W1rT f16
        P * 8 * 2,              # Wzo f16
        P * 2 * 4,              # b1p f32
        P * 2 * 4,              # b2b f32
        P * NB * 4,             # recip f32
    ]
    offs = []
    off = 0
    for s in sizes:
        offs.append(off)
        off += s + ((-(off + s)) % 256)
    return offs, off


def _build(W, sstep):
    C1 = NB * W
    nc = bacc.Bacc(None, target_bir_lowering=False, debug=False)

    x_q_d = nc.dram_tensor("x_q", [ROWS // 2, 2 * IN_C], i8, kind="ExternalInput")
    offs, SZ = _blob_offsets(W)
    tb_d = nc.dram_tensor("tb", [1, SZ], u8, kind="ExternalInput")
    out_d = nc.dram_tensor("out", [P, 2 * NB], f16, kind="ExternalOutput")
    bl = tb_d[0:1, :]

    def sec(i, dt, nelem, p):
        esz = mybir.dt.size(dt)
        v = bl[:, offs[i]:offs[i] + nelem * esz]
        if dt != u8:
            v = v.bitcast(dt)
        return v.rearrange("o (p c) -> (o p) c", p=p)

    idx_d = sec(0, i16, 16 * NB * 8 * W, 16)
    lpu_d = sec(1, u8, P * C1, P)
    sclu_d = sec(2, u8, P * C1, P)
    sclown_d = sec(3, f32, P * NB, P)
    W1lT_d = sec(4, f16, P * HID, P)
    W1rT_d = sec(5, f16, P * HID, P)
    Wzo_d = sec(6, f16, P * 8, P)
    b1p_d = sec(7, f32, P * 2, P)
    b2b_d = sec(8, f32, P * 2, P)
    recip_d = sec(9, f32, P * NB, P)

    with tile.TileContext(nc) as tc:
        with (
            tc.tile_pool(name="big", bufs=1) as big,
            tc.tile_pool(name="lp", bufs=4) as lp,
            tc.tile_pool(name="pp", bufs=2, space="PSUM") as pp,
            tc.tile_pool(name="dram", bufs=1, space="DRAM") as dp,
        ):
            def load(d, shape, dt, tag):
                t = big.tile(shape, dt, tag=tag, name=tag)
                nc.sync.dma_start(out=t[:], in_=d)
                return t

            W1lT_sb = load(W1lT_d, [P, HID], f16, "w1l")
            W1rT_sb = load(W1rT_d, [P, HID], f16, "w1r")
            Wzo_sb = load(Wzo_d, [P, 8], f16, "wzo")
            b1_sb = load(b1p_d, [P, 2], f32, "b1")
            b2_sb = load(b2b_d, [P, 2], f32, "b2")
            lpu_sb = load(lpu_d, [P, C1], u8, "lpu")
            sclu_sb = load(sclu_d, [P, C1], u8, "sclu")

            # replicate the 16-partition index block across all 8 core groups
            idx_sb = big.tile([P, NB * 8 * W], i16, tag="idx", name="idx_sb")
            for g in range(8):
                nc.sync.dma_start(
                    out=idx_sb[16 * g:16 * (g + 1), :], in_=idx_d
                )

            # widened tables: decode par*128+ldst byte; scale = code*sstep
            lpf = big.tile([P, C1], f32, tag="lpf", name="lpf")
            nc.vector.tensor_copy(out=lpf[:], in_=lpu_sb[:])
            parO = big.tile([P, C1], f32, tag="parO", name="parO")
            nc.vector.tensor_scalar(
                out=parO[:], in0=lpf[:], scalar1=128.0, scalar2=None,
                op0=mybir.AluOpType.is_ge,
            )
            ldst_sb = big.tile([P, C1], f32, tag="ldst", name="ldst_sb")
            nc.vector.scalar_tensor_tensor(
                out=ldst_sb[:], in0=parO[:], scalar=-128.0, in1=lpf[:],
                op0=mybir.AluOpType.mult, op1=mybir.AluOpType.add,
            )
            parE = big.tile([P, C1], f32, tag="parE", name="parE")
            nc.vector.tensor_scalar(
                out=parE[:], in0=parO[:], scalar1=-1.0, scalar2=1.0,
                op0=mybir.AluOpType.mult, op1=mybir.AluOpType.add,
            )
            scl = big.tile([P, C1], f32, tag="scl", name="scl")
            nc.vector.tensor_scalar(
                out=scl[:], in0=sclu_sb[:], scalar1=float(sstep), scalar2=None,
                op0=mybir.AluOpType.mult,
            )
            sclE = big.tile([P, C1], f32, tag="sclE", name="sclE")
            nc.vector.tensor_tensor(
                out=sclE[:], in0=scl[:], in1=parE[:], op=mybir.AluOpType.mult)
            sclO = big.tile([P, C1], f32, tag="sclO", name="sclO")
            nc.vector.tensor_tensor(
                out=sclO[:], in0=scl[:], in1=parO[:], op=mybir.AluOpType.mult)

            # iota / identity built on device
            ioti = big.tile([P, P], i32, tag="ioti", name="ioti")
            nc.gpsimd.iota(out=ioti[:], pattern=[[1, P]], base=0,
                           channel_multiplier=0)
            iotp = big.tile([P, P], i32, tag="iotp", name="iotp")
            nc.gpsimd.iota(out=iotp[:], pattern=[[0, P]], base=0,
                           channel_multiplier=1)
            iota_sb = big.tile([P, P], f32, tag="iota", name="iota_sb")
            nc.vector.tensor_copy(out=iota_sb[:], in_=ioti[:])
            identh = big.tile([P, P], f16, tag="identh", name="identh")
            nc.vector.tensor_tensor(
                out=identh[:], in0=ioti[:], in1=iotp[:],
                op=mybir.AluOpType.is_equal,
            )

            # x (int8, two nodes per 256B row) -> internal DRAM -> AllGather
            x_int = dp.tile([ROWS // 2, 2 * IN_C], i8, tag="xint", name="x_int")
            nc.sync.dma_start(out=x_int[:, :], in_=x_q_d[:, :])
            x_full = dp.tile([HNP + 1, 2 * IN_C], i8, tag="xfull",
                             name="x_full", addr_space="Shared")
            nc.gpsimd.collective_compute(
                "AllGather",
                mybir.AluOpType.bypass,
                replica_groups=[list(range(NCORES))],
                ins=[x_int[:, :]],
                outs=[x_full[0:HNP, :]],
            )
            xb_src = (x_int[:, :]
                      .rearrange("g (t c) -> (g t) c", t=2)
                      .rearrange("(p b) c -> p b c", b=NB))

            z_own = dp.tile([ROWS, 2], f16, tag="zown", name="z_own")
            z_own_v = z_own[:, :].rearrange("(p b) f -> p b f", b=NB)
            z_all = dp.tile([NP, 2], f16, tag="zall", name="z_all",
                            addr_space="Shared")
            z2 = dp.tile([HNP + 1, P], f16, tag="z2", name="z2")
            o_stage = dp.tile([P, 2 * NB], f32, tag="ostage", name="o_stage")

            out_sb = big.tile([P, 2 * NB], f32, tag="outs", name="out_sb")

            with tc.For_i(0, NB, name="l1") as b:
                g = lp.tile([P, W, 2 * IN_C], i8, tag="g", name="g")
                nc.gpsimd.dma_gather(
                    out_ap=g[:, :, :],
                    in_ap=x_full[:, :],
                    idxs_ap=idx_sb[:, ds(b * 8 * W, 8 * W)],
                    num_idxs=W * P,
                    num_idxs_reg=W * P,
                    elem_size=2 * IN_C,
                    single_packet=False,
                )
                gf = lp.tile([P, W, 2 * IN_C], f16, tag="gf", name="gf")
                nc.vector.tensor_copy(out=gf[:, :, :], in_=g[:, :, :])
                pagg = pp.tile([P, P], f32, tag="agg", name="pagg")
                for k in range(W):
                    PtE = lp.tile([P, P], f16, tag="P", name="PtE")
                    nc.vector.tensor_scalar(
                        out=PtE[:], in0=iota_sb[:],
                        scalar1=ldst_sb[:, ds(b * W + k, 1)],
                        scalar2=sclE[:, ds(b * W + k, 1)],
                        op0=mybir.AluOpType.is_equal, op1=mybir.AluOpType.mult,
                    )
                    nc.tensor.matmul(
                        out=pagg[:], lhsT=PtE[:], rhs=gf[:, k, 0:IN_C],
                        start=(k == 0), stop=False,
                    )
                    PtO = lp.tile([P, P], f16, tag="P", name="PtO")
                    nc.vector.tensor_scalar(
                        out=PtO[:], in0=iota_sb[:],
                        scalar1=ldst_sb[:, ds(b * W + k, 1)],
                        scalar2=sclO[:, ds(b * W + k, 1)],
                        op0=mybir.AluOpType.is_equal, op1=mybir.AluOpType.mult,
                    )
                    nc.tensor.matmul(
                        out=pagg[:], lhsT=PtO[:], rhs=gf[:, k, IN_C:2 * IN_C],
                        start=False, stop=(k == W - 1),
                    )
                rcb = lp.tile([P, 1], f32, tag="rcb", name="rcb")
                nc.sync.dma_start(out=rcb[:], in_=recip_d[:, ds(b, 1)])
                aggm = lp.tile([P, P], f16, tag="aggm", name="aggm")
                nc.vector.tensor_scalar(
                    out=aggm[:], in0=pagg[:], scalar1=rcb[:, 0:1],
                    scalar2=None, op0=mybir.AluOpType.mult,
                )
                ptr = pp.tile([P, P], f16, tag="tr", name="ptr", bufs=3)
                nc.tensor.transpose(out=ptr[:], in_=aggm[:], identity=identh[:])
                aggmT = lp.tile([P, P], f16, tag="aggmT", name="aggmT")
                nc.vector.tensor_copy(out=aggmT[:], in_=ptr[:])

                xb_q = lp.tile([P, IN_C], i8, tag="xbq", name="xb_q")
                nc.sync.dma_start(out=xb_q[:], in_=xb_src[:, ds(b, 1), :])
                scob = lp.tile([P, 1], f32, tag="scob", name="scob")
                nc.sync.dma_start(out=scob[:], in_=sclown_d[:, ds(b, 1)])
                xb = lp.tile([P, IN_C], f16, tag="xb", name="xb")
                nc.vector.tensor_scalar(
                    out=xb[:], in0=xb_q[:], scalar1=scob[:, 0:1],
                    scalar2=None, op0=mybir.AluOpType.mult,
                )
                ptr2 = pp.tile([P, P], f16, tag="tr", name="ptr2", bufs=3)
                nc.tensor.transpose(out=ptr2[:], in_=xb[:], identity=identh[:])
                xbT = lp.tile([P, P], f16, tag="xbT", name="xbT")
                nc.vector.tensor_copy(out=xbT[:], in_=ptr2[:])

                hbT = []
                for j in range(2):
                    ph = pp.tile([P, P], f32, tag="tr", name="ph", bufs=3)
                    nc.tensor.matmul(
                        out=ph[:], lhsT=W1lT_sb[:, j * P:(j + 1) * P],
                        rhs=aggmT[:], start=True, stop=False,
                    )
                    nc.tensor.matmul(
                        out=ph[:], lhsT=W1rT_sb[:, j * P:(j + 1) * P],
                        rhs=xbT[:], start=False, stop=True,
                    )
                    ht = lp.tile([P, P], f16, tag=f"hbT{j}", name=f"ht{j}")
                    nc.scalar.activation(
                        out=ht[:], in_=ph[:],
                        func=mybir.ActivationFunctionType.Relu,
                        bias=b1_sb[:, j:j + 1],
                    )
                    hbT.append(ht)
                pzo = pp.tile([P, 4], f32, tag="zo", name="pzo", bufs=1)
                for j in range(2):
                    nc.tensor.matmul(
                        out=pzo[:], lhsT=hbT[j][:],
                        rhs=Wzo_sb[:, 4 * j:4 * j + 4],
                        start=(j == 0), stop=(j == 1),
                    )
                zb = lp.tile([P, 2], f16, tag="zb", name="zb")
                nc.vector.tensor_copy(out=zb[:], in_=pzo[:, 0:2])
                nc.sync.dma_start(out=z_own_v[:, ds(b, 1), :], in_=zb[:])
                ob = lp.tile([P, 2], f32, tag="ob", name="ob")
                nc.vector.tensor_tensor(
                    out=ob[:], in0=pzo[:, 2:4], in1=b2_sb[:],
                    op=mybir.AluOpType.add,
                )
                nc.sync.dma_start(out=o_stage[:, ts(b, 2)], in_=ob[:])

            nc.gpsimd.collective_compute(
                "AllGather",
                mybir.AluOpType.bypass,
                replica_groups=[list(range(NCORES))],
                ins=[z_own[:, :]],
                outs=[z_all[:, :]],
            )
            # pack z pairs into 256B rows: z2[g, 0:4] = [z(2g) | z(2g+1)]
            nc.sync.dma_start(
                out=z2[0:HNP, 0:4],
                in_=z_all[:, :].rearrange("(g t) f -> g (t f)", t=2),
            )
            zpad = big.tile([1, 4], f16, tag="zpad", name="zpad")
            nc.vector.memset(zpad[:], 0.0)
            nc.sync.dma_start(out=z2[HNP:HNP + 1, 0:4], in_=zpad[:])

            with tc.For_i(0, NB, name="l2") as b:
                zg = lp.tile([P, W, P], f16, tag="zg", name="zg")
                nc.gpsimd.dma_gather(
                    out_ap=zg[:, :, :],
                    in_ap=z2[:, :],
                    idxs_ap=idx_sb[:, ds(b * 8 * W, 8 * W)],
                    num_idxs=W * P,
                    num_idxs_reg=W * P,
                    elem_size=P,
                    single_packet=False,
                )
                pa2 = pp.tile([P, 2], f32, tag="agg2", name="pa2")
                for k in range(W):
                    P2E = lp.tile([P, P], f16, tag="P", name="P2E")
                    nc.vector.tensor_scalar(
                        out=P2E[:], in0=iota_sb[:],
                        scalar1=ldst_sb[:, ds(b * W + k, 1)],
                        scalar2=parE[:, ds(b * W + k, 1)],
                        op0=mybir.AluOpType.is_equal, op1=mybir.AluOpType.mult,
                    )
                    nc.tensor.matmul(
                        out=pa2[:], lhsT=P2E[:], rhs=zg[:, k, 0:2],
                        start=(k == 0), stop=False,
                    )
                    P2O = lp.tile([P, P], f16, tag="P", name="P2O")
                    nc.vector.tensor_scalar(
                        out=P2O[:], in0=iota_sb[:],
                        scalar1=ldst_sb[:, ds(b * W + k, 1)],
                        scalar2=parO[:, ds(b * W + k, 1)],
                        op0=mybir.AluOpType.is_equal, op1=mybir.AluOpType.mult,
                    )
                    nc.tensor.matmul(
                        out=pa2[:], lhsT=P2O[:], rhs=zg[:, k, 2:4],
                        start=False, stop=(k == W - 1),
                    )
                rcb = lp.tile([P, 1], f32, tag="rcb", name="rcb2")
                nc.sync.dma_start(out=rcb[:], in_=recip_d[:, ds(b, 1)])
                red2 = lp.tile([P, 2], f32, tag="red2", name="red2")
                nc.vector.tensor_scalar(
                    out=red2[:], in0=pa2[:], scalar1=rcb[:, 0:1],
                    scalar2=None, op0=mybir.AluOpType.mult,
                )
                ob = lp.tile([P, 2], f32, tag="ob", name="ob2")
                nc.sync.dma_start(out=ob[:], in_=o_stage[:, ts(b, 2)])
                outb = lp.tile([P, 2], f32, tag="outb", name="outb")
                nc.vector.tensor_tensor(
                    out=outb[:], in0=red2[:], in1=ob[:],
                    op=mybir.AluOpType.add,
                )
                nc.vector.tensor_copy(out=out_sb[:, ts(b, 2)], in_=outb[:])

            nc.gpsimd.dma_start(out=out_d[:, :], in_=out_sb[:])
    nc.compile()
    return nc


def _run(inputs, repeat=1):
    in_maps, W, sstep = _host_prep(**inputs)
    nc = _build(W, sstep)
    best = None
    for _ in range(repeat):
        t0 = time.perf_counter()
        res = run_bass_kernel_spmd(
            nc, [dict(m) for m in in_maps], core_ids=list(range(NCORES))
        )
        dt = time.perf_counter() - t0
        print(f"  spmd run: {dt:.3f}s", flush=True)
        best = dt if best is None else min(best, dt)
    outs = []
    for c in range(NCORES):
        a = res.results[c]["out"]  # [128, 98]; row p, col 2b+f = node p*49+b
        outs.append(a.reshape(ROWS, 2))
    full = np.concatenate(outs, axis=0)[:N]
    return full.astype(np.float32), best


def kernel(**inputs):
    out, _ = _run(inputs, repeat=1)
    return out


# revision 37
# speedup vs baseline: 1.3384x; 1.3384x over previous
"""GraphSAGE 2-layer fraud detector on 8 trn2 NeuronCores.

Strategy (dst-partitioned, matmul scatter, minimal host->device traffic):
  - The axon tunnel moves ~45MB/s, so wall time is dominated by (a) input
    bytes shipped per spmd call and (b) per-call re-lowering of the kernel
    BIR (proportional to instruction count). Each core receives ONLY its
    x shard, quantized to int8 with a per-node fp16 scale (0.8MB), plus
    compressed edge tables; x is AllGathered across cores on-device, and
    everything else (iota, identity, x^T blocks, the z table) is derived
    on-device. All loops are tc.For_i hardware loops, so the kernel is a
    few hundred instructions regardless of edge count.
  - Nodes padded to 50176 = 8 cores x 49 blocks x 128. Core c owns nodes
    [c*6272, (c+1)*6272). Within a core, dst block b holds the 128 nodes
    with local index p*49 + b (p = row in block), which makes the z tile a
    plain contiguous view of z rows in node order.
  - Per-edge work is driven by gpsimd.dma_gather: one instruction gathers
    a whole block's 256B rows from an HBM table into SBUF. Rows pack two
    consecutive nodes (int8 x: 2x128B; z: 2x2 fp16 values in a padded
    row), so indices are src>>1 and fit int16. The parity selection AND
    the int8 dequant scale are folded into the one-hot scatter matrices:
      agg = sum_k [(iota==ldst_k)*sclE_k].T @ q_even
                + [(iota==ldst_k)*sclO_k].T @ q_odd
    where sclE/sclO = scale[src] masked by src parity (one fused
    tensor_scalar builds each matrix). Layer 2 uses the SAME index/ldst
    tables with parity masks instead of scales.
  - z = h@W2l.T, o = h@W2r.T + b2 (aggregation commutes with the linear
    map, so layer 2 aggregates 2-wide z, not 256-wide h); out =
    recip*agg2 + o.
"""

import time

import numpy as np

import concourse.bass as bass
import concourse.mybir as mybir
import concourse.tile as tile
from concourse import bacc
from concourse.bass import ds, ts
from concourse.bass_utils import run_bass_kernel_spmd

# generate_dve_tables() is a pure function of (trn_type, ops) but is re-run
# from scratch inside every run_bass_kernel_spmd call (~0.3s of deepcopy per
# call via neuronx_cc_hook -> compile_bir_kernel -> get_walrus_args).
# Memoize the common (ops == {}) case; the returned dict[str, bytes] is only
# ever read (write_dve_dir copies it to disk), so sharing one instance is safe.
import concourse.bass_utils as _bass_utils
import concourse.dve_table_gen as _dtg

_DVE_TABLE_CACHE: dict = {}
_orig_generate_dve_tables = _dtg.generate_dve_tables


def _cached_generate_dve_tables(trn_type, ops, base_dir=None):
    if ops or base_dir is not None:
        return _orig_generate_dve_tables(trn_type, ops, base_dir)
    if trn_type not in _DVE_TABLE_CACHE:
        _DVE_TABLE_CACHE[trn_type] = _orig_generate_dve_tables(trn_type, ops)
    return _DVE_TABLE_CACHE[trn_type]


_bass_utils.generate_dve_tables = _cached_generate_dve_tables
_dtg.generate_dve_tables = _cached_generate_dve_tables

# neuronx_cc_hook is likewise a pure function of the serialized HLO (which
# embeds the BIR), but each run_bass_kernel_spmd call re-runs walrus + NEFF
# tar repacking (~70ms) because every call makes a fresh jax.jit closure.
# Memoize per HLO bytes; the cached value is an immutable (rc, bytes) tuple.
import concourse.bass2jax as _b2j

_NEFF_HOOK_CACHE: dict = {}
_orig_neuronx_cc_hook = _b2j.neuronx_cc_hook


def _cached_neuronx_cc_hook(code, code_format, platform_version, file_prefix):
    if b"bass_exec" not in code:
        return _orig_neuronx_cc_hook(code, code_format, platform_version,
                                     file_prefix)
    key = (code, code_format, str(platform_version))
    r = _NEFF_HOOK_CACHE.get(key)
    if r is None:
        r = _orig_neuronx_cc_hook(code, code_format, platform_version,
                                  file_prefix)
        _NEFF_HOOK_CACHE[key] = r
    return r


_b2j.neuronx_cc_hook = _cached_neuronx_cc_hook

# Let XLA reuse compiled executables across the per-call fresh jit closures
# (harmless no-op if the backend doesn't support serialization).
try:
    import jax as _jax

    _jax.config.update("jax_compilation_cache_dir", "/tmp/jax_comp_cache")
    _jax.config.update("jax_persistent_cache_min_compile_time_secs", 0.0)
    _jax.config.update("jax_persistent_cache_min_entry_size_bytes", 0)
except Exception:
    pass

N = 50000
E = 800000
IN_C = 128
HID = 256
OUT_C = 2
NCORES = 8
P = 128
NB = 49                 # dst blocks per core
ROWS = NB * P           # 6272 rows per core
NP = NCORES * ROWS      # 50176 padded nodes
HNP = NP // 2           # 25088 paired rows (int16-addressable)

f32 = mybir.dt.float32
f16 = mybir.dt.float16
i32 = mybir.dt.int32
i16 = mybir.dt.int16
i8 = mybir.dt.int8
u8 = mybir.dt.uint8


def _wrap16(flat):
    """dma_gather index layout: flat j -> [partition j%16, col j//16]."""
    return np.ascontiguousarray(flat.reshape(-1, 16).T)


def _host_prep(x, edge_index, W1l, b1, W1r, W2l, b2, W2r):
    src = edge_index[0].astype(np.int64)
    dst = edge_index[1].astype(np.int64)
    cnt = np.bincount(dst, minlength=NP)
    recip = (1.0 / np.maximum(cnt, 1)).astype(np.float32)

    # int8 quantization of x with per-node fp16 scale
    x = np.asarray(x, np.float32)
    absmax = np.abs(x).max(axis=1)
    s_node = (np.maximum(absmax, 1e-6) / 127.0).astype(np.float16)
    s_full = np.ones(NP, np.float16)
    s_full[:N] = s_node
    q = np.zeros((NP, IN_C), np.int8)
    q[:N] = np.clip(np.rint(x / s_node.astype(np.float32)[:, None]),
                    -127, 127).astype(np.int8)

    # dst sort key in block-layout space: node (core c, local r) sits in
    # block b = r % 49 at row p = r // 49 -> key = c*6272 + b*128 + p.
    c_ = dst // ROWS
    r_ = dst % ROWS
    key = c_ * ROWS + (r_ % NB) * P + (r_ // NB)
    order = np.argsort(key, kind="stable")
    s_src = src[order]
    s_key = key[order]

    block_starts = np.searchsorted(s_key, np.arange(0, NP + P, P))
    cnt_blk = block_starts[1:] - block_starts[:-1]
    W = int(np.maximum(1, -(-cnt_blk // P)).max())  # uniform chunks per block
    C1 = NB * W

    # scale codes: 0 = pad (kills the edge in layer 1), else s = code*sstep
    smax = float(s_full[:N].max()) if N else 1.0
    sstep = smax / 255.0
    idx_arr = np.zeros((NCORES, 16, NB * 8 * W), np.int16)
    lp_arr = np.zeros((NCORES, P, C1), np.uint8)   # par*128 + ldst
    scl_arr = np.zeros((NCORES, P, C1), np.uint8)
    for c in range(NCORES):
        for b in range(NB):
            bb = c * NB + b
            s, e = int(block_starts[bb]), int(block_starts[bb + 1])
            k = e - s
            bs = s_src[s:e]
            fi = np.full(W * P, HNP, np.int16)   # pad -> zero row
            fi[:k] = bs >> 1
            idx_arr[c, :, b * 8 * W:(b + 1) * 8 * W] = _wrap16(fi)
            tl = np.zeros(W * P, np.uint8)
            tl[:k] = ((s_key[s:e] % P)
                      + 128 * (bs & 1)).astype(np.uint8)
            lp_arr[c, :, b * W:(b + 1) * W] = tl.reshape(W, P).T
            tsc = np.zeros(W * P, np.uint8)
            tsc[:k] = np.maximum(
                1, np.rint(s_full[bs].astype(np.float64) / sstep)
            ).astype(np.uint8)
            scl_arr[c, :, b * W:(b + 1) * W] = tsc.reshape(W, P).T

    W1lT = np.ascontiguousarray(W1l.T.astype(np.float16))   # [128, 256]
    W1rT = np.ascontiguousarray(W1r.T.astype(np.float16))
    Wzo = np.zeros((P, 8), np.float16)
    for j in range(2):
        Wzo[:, 4 * j:4 * j + 2] = W2l.T[j * P:(j + 1) * P, :].astype(np.float16)
        Wzo[:, 4 * j + 2:4 * j + 4] = W2r.T[j * P:(j + 1) * P, :].astype(np.float16)
    b1p = np.ascontiguousarray(np.asarray(b1).reshape(2, P).T.astype(np.float32))
    b2b = np.tile(np.asarray(b2).reshape(1, 2), (P, 1)).astype(np.float32)
    recip_c = recip.reshape(NCORES, P, NB).copy()   # node local r = p*49+b
    s_own = s_full.astype(np.float32).reshape(NCORES, P, NB)

    in_maps = []
    for c in range(NCORES):
        sections = [
            np.ascontiguousarray(idx_arr[c]),
            np.ascontiguousarray(lp_arr[c]),
            np.ascontiguousarray(scl_arr[c]),
            np.ascontiguousarray(s_own[c]),
            W1lT, W1rT, Wzo, b1p, b2b,
            np.ascontiguousarray(recip_c[c]),
        ]
        parts = []
        off = 0
        for a in sections:
            bb_ = a.ravel().view(np.uint8)
            parts.append(bb_)
            off += len(bb_)
            pad = (-off) % 256
            if pad:
                parts.append(np.zeros(pad, np.uint8))
                off += pad
        in_maps.append({
            "x_q": np.ascontiguousarray(
                q[c * ROWS:(c + 1) * ROWS, :].reshape(ROWS // 2, 2 * IN_C)),
            "tb": np.concatenate(parts)[None, :],
        })
    return in_maps, W, sstep


def _blob_offsets(W):
    C1 = NB * W
    sizes = [
        16 * NB * 8 * W * 2,    # idx16
        P * C1,                 # lpu
        P * C1,                 # sclu
        P * NB * 4,             # sclown f32
        P * HID * 2,            # W1lT f16
        P * HID * 2,            # W1rT f16
        P * 8 * 2,              # Wzo f16
        P * 2 * 4,              # b1p f32
        P * 2 * 4,              # b2b f32
        P * NB * 4,             # recip f32
    ]
    offs = []
    off = 0
    for s in sizes:
        offs.append(off)
        off += s + ((-(off + s)) % 256)
    return offs, off


def _build(W, sstep):
    C1 = NB * W
    nc = bacc.Bacc(None, target_bir_lowering=False, debug=False)

    x_q_d = nc.dram_tensor("x_q", [ROWS // 2, 2 * IN_C], i8, kind="ExternalInput")
    offs, SZ = _blob_offsets(W)
    tb_d = nc.dram_tensor("tb", [1, SZ], u8, kind="ExternalInput")
    out_d = nc.dram_tensor("out", [P, 2 * NB], f16, kind="ExternalOutput")
    bl = tb_d[0:1, :]

    def sec(i, dt, nelem, p):
        esz = mybir.dt.size(dt)
        v = bl[:, offs[i]:offs[i] + nelem * esz]
        if dt != u8:
            v = v.bitcast(dt)
        return v.rearrange("o (p c) -> (o p) c", p=p)

    idx_d = sec(0, i16, 16 * NB * 8 * W, 16)
    lpu_d = sec(1, u8, P * C1, P)
    sclu_d = sec(2, u8, P * C1, P)
    sclown_d = sec(3, f32, P * NB, P)
    W1lT_d = sec(4, f16, P * HID, P)
    W1rT_d = sec(5, f16, P * HID, P)
    Wzo_d = sec(6, f16, P * 8, P)
    b1p_d = sec(7, f32, P * 2, P)
    b2b_d = sec(8, f32, P * 2, P)
    recip_d = sec(9, f32, P * NB, P)

    with tile.TileContext(nc) as tc:
        with (
            tc.tile_pool(name="big", bufs=1) as big,
            tc.tile_pool(name="lp", bufs=4) as lp,
            tc.tile_pool(name="pp", bufs=2, space="PSUM") as pp,
            tc.tile_pool(name="dram", bufs=1, space="DRAM") as dp,
        ):
            def load(d, shape, dt, tag):
                t = big.tile(shape, dt, tag=tag, name=tag)
                nc.sync.dma_start(out=t[:], in_=d)
                return t

            W1lT_sb = load(W1lT_d, [P, HID], f16, "w1l")
            W1rT_sb = load(W1rT_d, [P, HID], f16, "w1r")
            Wzo_sb = load(Wzo_d, [P, 8], f16, "wzo")
            b1_sb = load(b1p_d, [P, 2], f32, "b1")
            b2_sb = load(b2b_d, [P, 2], f32, "b2")
            lpu_sb = load(lpu_d, [P, C1], u8, "lpu")
            sclu_sb = load(sclu_d, [P, C1], u8, "sclu")

            # replicate the 16-partition index block across all 8 core groups
            idx_sb = big.tile([P, NB * 8 * W], i16, tag="idx", name="idx_sb")
            for g in range(8):
                nc.sync.dma_start(
                    out=idx_sb[16 * g:16 * (g + 1), :], in_=idx_d
                )

            # widened tables: decode par*128+ldst byte; scale = code*sstep
            lpf = big.tile([P, C1], f32, tag="lpf", name="lpf")
            nc.vector.tensor_copy(out=lpf[:], in_=lpu_sb[:])
            parO = big.tile([P, C1], f32, tag="parO", name="parO")
            nc.vector.tensor_scalar(
                out=parO[:], in0=lpf[:], scalar1=128.0, scalar2=None,
                op0=mybir.AluOpType.is_ge,
            )
            ldst_sb = big.tile([P, C1], f32, tag="ldst", name="ldst_sb")
            nc.vector.scalar_tensor_tensor(
                out=ldst_sb[:], in0=parO[:], scalar=-128.0, in1=lpf[:],
                op0=mybir.AluOpType.mult, op1=mybir.AluOpType.add,
            )
            parE = big.tile([P, C1], f32, tag="parE", name="parE")
            nc.vector.tensor_scalar(
                out=parE[:], in0=parO[:], scalar1=-1.0, scalar2=1.0,
                op0=mybir.AluOpType.mult, op1=mybir.AluOpType.add,
            )
            scl = big.tile([P, C1], f32, tag="scl", name="scl")
            nc.vector.tensor_scalar(
                out=scl[:], in0=sclu_sb[:], scalar1=float(sstep), scalar2=None,
                op0=mybir.AluOpType.mult,
            )
            sclE = big.tile([P, C1], f32, tag="sclE", name="sclE")
            nc.vector.tensor_tensor(
                out=sclE[:], in0=scl[:], in1=parE[:], op=mybir.AluOpType.mult)
            sclO = big.tile([P, C1], f32, tag="sclO", name="sclO")
            nc.vector.tensor_tensor(
                out=sclO[:], in0=scl[:], in1=parO[:], op=mybir.AluOpType.mult)

            # iota / identity built on device
            ioti = big.tile([P, P], i32, tag="ioti", name="ioti")
            nc.gpsimd.iota(out=ioti[:], pattern=[[1, P]], base=0,
                           channel_multiplier=0)
            iotp = big.tile([P, P], i32, tag="iotp", name="iotp")
            nc.gpsimd.iota(out=iotp[:], pattern=[[0, P]], base=0,
                           channel_multiplier=1)
            iota_sb = big.tile([P, P], f32, tag="iota", name="iota_sb")
            nc.vector.tensor_copy(out=iota_sb[:], in_=ioti[:])
            identh = big.tile([P, P], f16, tag="identh", name="identh")
            nc.vector.tensor_tensor(
                out=identh[:], in0=ioti[:], in1=iotp[:],
                op=mybir.AluOpType.is_equal,
            )

            # x (int8, two nodes per 256B row) -> internal DRAM -> AllGather
            x_int = dp.tile([ROWS // 2, 2 * IN_C], i8, tag="xint", name="x_int")
            nc.sync.dma_start(out=x_int[:, :], in_=x_q_d[:, :])
            x_full = dp.tile([HNP + 1, 2 * IN_C], i8, tag="xfull",
                             name="x_full", addr_space="Shared")
            nc.gpsimd.collective_compute(
                "AllGather",
                mybir.AluOpType.bypass,
                replica_groups=[list(range(NCORES))],
                ins=[x_int[:, :]],
                outs=[x_full[0:HNP, :]],
            )
            xb_src = (x_int[:, :]
                      .rearrange("g (t c) -> (g t) c", t=2)
                      .rearrange("(p b) c -> p b c", b=NB))

            z_own = dp.tile([ROWS, 2], f16, tag="zown", name="z_own")
            z_own_v = z_own[:, :].rearrange("(p b) f -> p b f", b=NB)
            z_all = dp.tile([NP, 2], f16, tag="zall", name="z_all",
                            addr_space="Shared")
            z2 = dp.tile([HNP + 1, P], f16, tag="z2", name="z2")
            o_stage = dp.tile([P, 2 * NB], f32, tag="ostage", name="o_stage")

            out_sb = big.tile([P, 2 * NB], f32, tag="outs", name="out_sb")

            with tc.For_i(0, NB, name="l1") as b:
                g = lp.tile([P, W, 2 * IN_C], i8, tag="g", name="g")
                nc.gpsimd.dma_gather(
                    out_ap=g[:, :, :],
                    in_ap=x_full[:, :],
                    idxs_ap=idx_sb[:, ds(b * 8 * W, 8 * W)],
                    num_idxs=W * P,
                    num_idxs_reg=W * P,
                    elem_size=2 * IN_C,
                    single_packet=False,
                )
                gf = lp.tile([P, W, 2 * IN_C], f16, tag="gf", name="gf")
                nc.vector.tensor_copy(out=gf[:, :, :], in_=g[:, :, :])
                pagg = pp.tile([P, P], f32, tag="agg", name="pagg")
                for k in range(W):
                    PtE = lp.tile([P, P], f16, tag="P", name="PtE")
                    nc.vector.tensor_scalar(
                        out=PtE[:], in0=iota_sb[:],
                        scalar1=ldst_sb[:, ds(b * W + k, 1)],
                        scalar2=sclE[:, ds(b * W + k, 1)],
                        op0=mybir.AluOpType.is_equal, op1=mybir.AluOpType.mult,
                    )
                    nc.tensor.matmul(
                        out=pagg[:], lhsT=PtE[:], rhs=gf[:, k, 0:IN_C],
                        start=(k == 0), stop=False,
                    )
                    PtO = lp.tile([P, P], f16, tag="P", name="PtO")
                    nc.vector.tensor_scalar(
                        out=PtO[:], in0=iota_sb[:],
                        scalar1=ldst_sb[:, ds(b * W + k, 1)],
                        scalar2=sclO[:, ds(b * W + k, 1)],
                        op0=mybir.AluOpType.is_equal, op1=mybir.AluOpType.mult,
                    )
                    nc.tensor.matmul(
                        out=pagg[:], lhsT=PtO[:], rhs=gf[:, k, IN_C:2 * IN_C],
                        start=False, stop=(k == W - 1),
                    )
                rcb = lp.tile([P, 1], f32, tag="rcb", name="rcb")
                nc.sync.dma_start(out=rcb[:], in_=recip_d[:, ds(b, 1)])
                aggm = lp.tile([P, P], f16, tag="aggm", name="aggm")
                nc.vector.tensor_scalar(
                    out=aggm[:], in0=pagg[:], scalar1=rcb[:, 0:1],
                    scalar2=None, op0=mybir.AluOpType.mult,
                )
                ptr = pp.tile([P, P], f16, tag="tr", name="ptr", bufs=3)
                nc.tensor.transpose(out=ptr[:], in_=aggm[:], identity=identh[:])
                aggmT = lp.tile([P, P], f16, tag="aggmT", name="aggmT")
                nc.vector.tensor_copy(out=aggmT[:], in_=ptr[:])

                xb_q = lp.tile([P, IN_C], i8, tag="xbq", name="xb_q")
                nc.sync.dma_start(out=xb_q[:], in_=xb_src[:, ds(b, 1), :])
                scob = lp.tile([P, 1], f32, tag="scob", name="scob")
                nc.sync.dma_start(out=scob[:], in_=sclown_d[:, ds(b, 1)])
                xb = lp.tile([P, IN_C], f16, tag="xb", name="xb")
                nc.vector.tensor_scalar(
                    out=xb[:], in0=xb_q[:], scalar1=scob[:, 0:1],
                    scalar2=None, op0=mybir.AluOpType.mult,
                )
                ptr2 = pp.tile([P, P], f16, tag="tr", name="ptr2", bufs=3)
                nc.tensor.transpose(out=ptr2[:], in_=xb[:], identity=identh[:])
                xbT = lp.tile([P, P], f16, tag="xbT", name="xbT")
                nc.vector.tensor_copy(out=xbT[:], in_=ptr2[:])

                hbT = []
                for j in range(2):
                    ph = pp.tile([P, P], f32, tag="tr", name="ph", bufs=3)
                    nc.tensor.matmul(
                        out=ph[:], lhsT=W1lT_sb[:, j * P:(j + 1) * P],
                        rhs=aggmT[:], start=True, stop=False,
                    )
                    nc.tensor.matmul(
                        out=ph[:], lhsT=W1rT_sb[:, j * P:(j + 1) * P],
                        rhs=xbT[:], start=False, stop=True,
                    )
                    ht = lp.tile([P, P], f16, tag=f"hbT{j}", name=f"ht{j}")
                    nc.scalar.activation(
                        out=ht[:], in_=ph[:],
                        func=mybir.ActivationFunctionType.Relu,
                        bias=b1_sb[:, j:j + 1],
                    )
                    hbT.append(ht)
                pzo = pp.tile([P, 4], f32, tag="zo", name="pzo", bufs=1)
                for j in range(2):
                    nc.tensor.matmul(
                        out=pzo[:], lhsT=hbT[j][:],
                        rhs=Wzo_sb[:, 4 * j:4 * j + 4],
                        start=(j == 0), stop=(j == 1),
                    )
                zb = lp.tile([P, 2], f16, tag="zb", name="zb")
                nc.vector.tensor_copy(out=zb[:], in_=pzo[:, 0:2])
                nc.sync.dma_start(out=z_own_v[:, ds(b, 1), :], in_=zb[:])
                ob = lp.tile([P, 2], f32, tag="ob", name="ob")
                nc.vector.tensor_tensor(
                    out=ob[:], in0=pzo[:, 2:4], in1=b2_sb[:],
                    op=mybir.AluOpType.add,
                )
                nc.sync.dma_start(out=o_stage[:, ts(b, 2)], in_=ob[:])

            nc.gpsimd.collective_compute(
                "AllGather",
                mybir.AluOpType.bypass,
                replica_groups=[list(range(NCORES))],
                ins=[z_own[:, :]],
                outs=[z_all[:, :]],
            )
            # pack z pairs into 256B rows: z2[g, 0:4] = [z(2g) | z(2g+1)]
            nc.sync.dma_start(
                out=z2[0:HNP, 0:4],
                in_=z_all[:, :].rearrange("(g t) f -> g (t f)", t=2),
            )
            zpad = big.tile([1, 4], f16, tag="zpad", name="zpad")
            nc.vector.memset(zpad[:], 0.0)
            nc.sync.dma_start(out=z2[HNP:HNP + 1, 0:4], in_=zpad[:])

            with tc.For_i(0, NB, name="l2") as b:
                zg = lp.tile([P, W, P], f16, tag="zg", name="zg")
                nc.gpsimd.dma_gather(
                    out_ap=zg[:, :, :],
                    in_ap=z2[:, :],
                    idxs_ap=idx_sb[:, ds(b * 8 * W, 8 * W)],
                    num_idxs=W * P,
                    num_idxs_reg=W * P,
                    elem_size=P,
                    single_packet=False,
                )
                pa2 = pp.tile([P, 2], f32, tag="agg2", name="pa2")
                for k in range(W):
                    P2E = lp.tile([P, P], f16, tag="P", name="P2E")
                    nc.vector.tensor_scalar(
                        out=P2E[:], in0=iota_sb[:],
                        scalar1=ldst_sb[:, ds(b * W + k, 1)],
                        scalar2=parE[:, ds(b * W + k, 1)],
                        op0=mybir.AluOpType.is_equal, op1=mybir.AluOpType.mult,
                    )
                    nc.tensor.matmul(
                        out=pa2[:], lhsT=P2E[:], rhs=zg[:, k, 0:2],
                        start=(k == 0), stop=False,
                    )
                    P2O = lp.tile([P, P], f16, tag="P", name="P2O")
                    nc.vector.tensor_scalar(
                        out=P2O[:], in0=iota_sb[:],
                        scalar1=ldst_sb[:, ds(b * W + k, 1)],
                        scalar2=parO[:, ds(b * W + k, 1)],
                        op0=mybir.AluOpType.is_equal, op1=mybir.AluOpType.mult,
                    )
                    nc.tensor.matmul(
                        out=pa2[:], lhsT=P2O[:], rhs=zg[:, k, 2:4],
                        start=False, stop=(k == W - 1),
                    )
                rcb = lp.tile([P, 1], f32, tag="rcb", name="rcb2")
                nc.sync.dma_start(out=rcb[:], in_=recip_d[:, ds(b, 1)])
                red2 = lp.tile([P, 2], f32, tag="red2", name="red2")
                nc.vector.tensor_scalar(
                    out=red2[:], in0=pa2[:], scalar1=rcb[:, 0:1],
                    scalar2=None, op0=mybir.AluOpType.mult,
                )
                ob = lp.tile([P, 2], f32, tag="ob", name="ob2")
                nc.sync.dma_start(out=ob[:], in_=o_stage[:, ts(b, 2)])
                outb = lp.tile([P, 2], f32, tag="outb", name="outb")
                nc.vector.tensor_tensor(
                    out=outb[:], in0=red2[:], in1=ob[:],
                    op=mybir.AluOpType.add,
                )
                nc.vector.tensor_copy(out=out_sb[:, ts(b, 2)], in_=outb[:])

            nc.gpsimd.dma_start(out=out_d[:, :], in_=out_sb[:])
    nc.compile()
    return nc


def _run(inputs, repeat=1):
    in_maps, W, sstep = _host_prep(**inputs)
    nc = _build(W, sstep)
    best = None
    for _ in range(repeat):
        t0 = time.perf_counter()
        res = run_bass_kernel_spmd(
            nc, [dict(m) for m in in_maps], core_ids=list(range(NCORES))
        )
        dt = time.perf_counter() - t0
        print(f"  spmd run: {dt:.3f}s", flush=True)
        best = dt if best is None else min(best, dt)
    outs = []
    for c in range(NCORES):
        a = res.results[c]["out"]  # [128, 98]; row p, col 2b+f = node p*49+b
        outs.append(a.reshape(ROWS, 2))
    full = np.concatenate(outs, axis=0)[:N]
    return full.astype(np.float32), best


def kernel(**inputs):
    out, _ = _run(inputs, repeat=1)
    return out
